# revision 7
# baseline (speedup 1.0000x reference)
"""Causal self-attention (GQA + qk RMS-norm + RoPE + q-gain) TRN2 Bass kernel.

Sharding: 8 cores = 2 batches x 4 kv-groups. Core c -> (b = c // 4,
g = c % 4): q heads 4g..4g+3, kv head g. Each core computes a partial
projection output (full [S, D], fp16); host sums the 4 partials per batch
in f32 (the unshard step for input-dim-sharded wproj).

Per-core program, fully 16-bit datapath (PSUM accumulation in f32):
  B  QKV projections (Q and KV interleaved per k-tile to share the
     stationary x weight load), RMS-norm stats/scales on fp16 PSUM
     evacuations, per-t-tile RoPE, PE transposes packed 5-to-a-bank and
     deferred two t-tiles so their rope chain is done (no PE stall).
  C  attention per (q-chunk, head-pair): one kT/v weight load per key
     tile feeds both heads; exp on ACT into a shared per-pair fp16 tile,
     skipping fully-masked column prefixes in diagonal blocks; running
     sum of exp tiles on DVE replaces per-tile denominator matmuls (one
     [1,512] matmul per head at the end); 1/den broadcast via gpsimd
     partition_broadcast.
  D  output projection units interleaved into the next q-chunk's
     attention loop so the in-order PE queue stays fed while ACT runs
     exp; results stream out over the sync/HWDGE queue in fp16.
"""
import sys

sys.path.insert(0, "/opt/trn_rl_repo")

from contextlib import ExitStack

import numpy as np

import concourse.bacc as bacc
import concourse.tile as tile
import concourse.mybir as mybir

F32 = mybir.dt.float32
F32R = mybir.dt.float32r
FP16 = mybir.dt.float16

S = 2048
D = 2048
HD = 128
NH_CORE = 4  # q heads per core
ROPE_BASE = 10000.0
EPS = 1.1920929e-07
NT = S // 128  # 16 t-tiles
NG = 4  # groups of 4 t-tiles
MASKVAL = -1e30
ALU = mybir.AluOpType
AF = mybir.ActivationFunctionType


def build_program(num_devices=8, phases="BCD"):
    nc = bacc.Bacc("TRN2", target_bir_lowering=False, debug=False,
                   num_devices=num_devices)

    xT = nc.dram_tensor("xT", (D, S), FP16, kind="ExternalInput").ap()
    wq = nc.dram_tensor("wq", (D, 512), FP16, kind="ExternalInput").ap()
    wkv = nc.dram_tensor("wkv", (D, 256), FP16, kind="ExternalInput").ap()
    wproj = nc.dram_tensor("wproj", (512, D), FP16, kind="ExternalInput").ap()
    c2d = nc.dram_tensor("c2", (S, 128), FP16, kind="ExternalInput").ap()
    s2d = nc.dram_tensor("s2", (S, 128), FP16, kind="ExternalInput").ap()
    gainsd = nc.dram_tensor("gains", (128, 4), F32, kind="ExternalInput").ap()
    keepwd = nc.dram_tensor("keepw", (128, 896), FP16, kind="ExternalInput").ap()
    identd = nc.dram_tensor("ident", (128, 128), FP16, kind="ExternalInput").ap()
    onescd = nc.dram_tensor("onescol", (128, 1), FP16, kind="ExternalInput").ap()
    onesrd = nc.dram_tensor("onesrow", (1, 128), FP16, kind="ExternalInput").ap()
    outd = nc.dram_tensor("out", (S, D), FP16, kind="ExternalOutput").ap()

    with tile.TileContext(nc) as tc, ExitStack() as ctx:
        # ---------------- persistent pools ----------------
        sbc = ctx.enter_context(tc.tile_pool(name="consts", bufs=1))
        qtp = ctx.enter_context(tc.tile_pool(name="qtp", bufs=4))

        # ---------------- constants / weights resident ----------------
        wq_sbs, wkv_sbs = [], []
        for k in range(0, 16, 2):
            wqt = sbc.tile([128, 2 * 512], FP16, tag=f"wq{k}",
                           name=f"wq_sb_{k}")
            nc.gpsimd.dma_start(
                wqt[:].rearrange("p (k o) -> p k o", k=2),
                wq[k * 128:(k + 2) * 128, :].rearrange(
                    "(k p) o -> p k o", p=128),
            )
            wq_sbs.append(wqt)
            wkt = sbc.tile([128, 2 * 256], FP16, tag=f"wkv{k}",
                           name=f"wkv_sb_{k}")
            nc.gpsimd.dma_start(
                wkt[:].rearrange("p (k o) -> p k o", k=2),
                wkv[k * 128:(k + 2) * 128, :].rearrange(
                    "(k p) o -> p k o", p=128),
            )
            wkv_sbs.append(wkt)
        c2_sb = sbc.tile([128, 16 * 128], FP16, tag="c2")
        nc.gpsimd.dma_start(
            c2_sb[:].rearrange("p (i d) -> p i d", i=16),
            c2d.rearrange("(i p) d -> p i d", p=128),
        )
        s2_sb = sbc.tile([128, 16 * 128], FP16, tag="s2")
        nc.gpsimd.dma_start(
            s2_sb[:].rearrange("p (i d) -> p i d", i=16),
            s2d.rearrange("(i p) d -> p i d", p=128),
        )
        gains_sb = sbc.tile([128, 4], F32, tag="gains")
        nc.gpsimd.dma_start(gains_sb[:], gainsd)
        keepw_sb = sbc.tile([128, 896], FP16, tag="keepw")
        nc.gpsimd.dma_start(keepw_sb[:], keepwd)
        ident_sb = sbc.tile([128, 128], FP16, tag="ident")
        nc.gpsimd.dma_start(ident_sb[:], identd)
        onesc_sb = sbc.tile([128, 1], FP16, tag="onesc")
        nc.gpsimd.dma_start(onesc_sb[:], onescd)
        onesr_sb = sbc.tile([1, 128], FP16, tag="onesr")
        nc.gpsimd.dma_start(onesr_sb[:], onesrd)

        wpp = ctx.enter_context(tc.tile_pool(name="wpp", bufs=16))
        wp = {}
        for h in range(NH_CORE):
            for dc in range(4):
                w = wpp.tile([128, 512], FP16, tag="wp", name=f"wp_{h}_{dc}")
                nc.gpsimd.dma_start(
                    w[:],
                    wproj[h * 128:(h + 1) * 128, dc * 512:(dc + 1) * 512],
                )
                wp[(h, dc)] = w

        negb_sb = sbc.tile([128, 1], F32, tag="negb")
        nc.vector.memset(negb_sb[:], -1.0)
        kTg = [sbc.tile([128, 512], FP16, tag=f"kT{g}", name=f"kTg_{g}")
               for g in range(4)]
        vg = [sbc.tile([128, 512], FP16, tag=f"v{g}", name=f"vg_{g}")
              for g in range(4)]

        qT = {}   # (h, g) -> [128 d, 512 t] tile
        ytile = {}  # (h, qc) -> [128 d, 512 t] tile

        # ================ phase B: projections + norm + rope + transpose ====
        ctxB = ExitStack()
        io2k = ctxB.enter_context(tc.tile_pool(name="io2k", bufs=17))
        work = ctxB.enter_context(tc.tile_pool(name="work", bufs=1))
        qfp = ctxB.enter_context(tc.tile_pool(name="qfp", bufs=1))
        smp = ctxB.enter_context(tc.tile_pool(name="smp", bufs=4))
        psQ = ctxB.enter_context(tc.tile_pool(name="psQ", bufs=3, space="PSUM"))
        psKV = ctxB.enter_context(tc.tile_pool(name="psKV", bufs=2, space="PSUM"))
        psTR = ctxB.enter_context(tc.tile_pool(name="psTR", bufs=3, space="PSUM"))

        def emit_transposes(st):
            # all 5 transposes of one t-tile share one PSUM bank; ACT
            # evacuates q into the group tile (layout [tt, h, d]) and k
            # into kT_sb
            g, tt, i, qf, kf, qtg = st
            trp = psTR.tile([128, 640], FP16, tag="pTR", name=f"tr_{i}")
            for h in range(NH_CORE):
                nc.tensor.transpose(
                    trp[:, h * 128:(h + 1) * 128],
                    qf[:, tt * 512 + h * 128: tt * 512 + (h + 1) * 128],
                    ident_sb[:],
                )
            nc.tensor.transpose(
                trp[:, 512:640],
                kf[:, tt * 128:(tt + 1) * 128],
                ident_sb[:],
            )
            nc.scalar.copy(qtg[:, tt * 512:(tt + 1) * 512], trp[:, 0:512])
            nc.scalar.copy(kTg[g][:, tt * 128:(tt + 1) * 128],
                           trp[:, 512:640])

        from collections import deque
        trpend = deque()
        for g in range(NG):
            xts = []
            for k in range(16):
                xt = io2k.tile([128, 512], FP16, tag="io", name=f"xt_{g}_{k}")
                nc.sync.dma_start(
                    xt[:], xT[k * 128:(k + 1) * 128, g * 512:(g + 1) * 512]
                )
                xts.append(xt)
            xts = [t[:] for t in xts]

            qn = work.tile([128, 2048], FP16, tag="qn", name=f"qn_{g}", bufs=2)
            kn = work.tile([128, 512], FP16, tag="kn", name=f"kn_{g}", bufs=2)
            sq = work.tile([128, 512], FP16, tag="sq", name=f"sq_{g}", bufs=2)
            qf = qfp.tile([128, 2048], FP16, tag="qf", name=f"qf_{g}", bufs=2)
            kf = work.tile([128, 512], FP16, tag="kf", name=f"kf_{g}", bufs=2)
            kt1 = work.tile([128, 512], FP16, tag="kt", name=f"kt1_{g}", bufs=2)
            t1 = work.tile([128, 2048], FP16, tag="rt", name=f"t1_{g}", bufs=2)
            qtg = qtp.tile([128, 2048], FP16, tag="qT", name=f"qtg_{g}")
            for h in range(NH_CORE):
                qT[(h, g)] = qtg
            for tt in range(4):
                i = g * 4 + tt
                # interleave Q and KV matmuls per k so both share one
                # weight load of the same x tile (stationary operand)
                psq = psQ.tile([128, 512], F32, tag="pQ", name=f"psq_{i}")
                pskv = psKV.tile([128, 256], F32, tag="pKV", name=f"pskv_{i}")
                for k in range(16):
                    nc.tensor.matmul(
                        psq[:],
                        xts[k][:, tt * 128:(tt + 1) * 128],
                        wq_sbs[k // 2][:, (k % 2) * 512:
                                       (k % 2 + 1) * 512],
                        start=(k == 0), stop=(k == 15),
                    )
                    nc.tensor.matmul(
                        pskv[:],
                        xts[k][:, tt * 128:(tt + 1) * 128],
                        wkv_sbs[k // 2][:, (k % 2) * 256:
                                        (k % 2 + 1) * 256],
                        start=(k == 0), stop=(k == 15),
                    )
                # transposes run two t-tiles behind so their rope
                # (a 3-engine chain) is surely done -- no PE stall
                if len(trpend) >= 2:
                    emit_transposes(trpend.popleft())

                # evacuate to fp16 (PSUM allows only one non-scalar read
                # per op, so stats can't read PSUM twice); fp16 halves the
                # DVE cost of stats/scales vs the f32 path
                q16 = smp.tile([128, 512], FP16, tag="q16",
                               name=f"q16_{i}", bufs=2)
                nc.vector.tensor_copy(q16[:], psq[:])
                kv16 = smp.tile([128, 256], FP16, tag="kv16",
                                name=f"kv16_{i}", bufs=2)
                nc.scalar.copy(kv16[:], pskv[:])
                ms = smp.tile([128, 5], F32, tag="ms", name=f"ms_{i}")
                for h in range(NH_CORE):
                    nc.vector.scalar_tensor_tensor(
                        out=sq[:, h * 128:(h + 1) * 128],
                        in0=q16[:, h * 128:(h + 1) * 128],
                        scalar=1.0,
                        in1=q16[:, h * 128:(h + 1) * 128],
                        op0=ALU.mult, op1=ALU.mult,
                        accum_out=ms[:, h:h + 1],
                    )
                nc.vector.scalar_tensor_tensor(
                    out=sq[:, 384:512],
                    in0=kv16[:, 0:128], scalar=1.0, in1=kv16[:, 0:128],
                    op0=ALU.mult, op1=ALU.mult,
                    accum_out=ms[:, 4:5],
                )
                msx = smp.tile([128, 5], F32, tag="msx", name=f"msx_{i}")
                nc.vector.tensor_scalar(msx[:], ms[:], 1.0 / HD, EPS,
                                        op0=ALU.mult, op1=ALU.add)
                u = smp.tile([128, 5], F32, tag="u", name=f"u_{i}")
                usc = smp.tile([128, 5], F32, tag="usc", name=f"usc_{i}")
                nc.vector.reciprocal_approx_accurate(out=u[:], in_=msx[:],
                                                     scratch=usc[:])
                rin = smp.tile([128, 5], F32, tag="rin", name=f"rin_{i}")
                nc.scalar.activation(rin[:], u[:], AF.Sqrt)
                ring = smp.tile([128, 4], F32, tag="ring", name=f"ring_{i}")
                nc.vector.tensor_mul(ring[:], rin[:, 0:4], gains_sb[:])

                # scale into qn / kn, copy v
                for h in range(NH_CORE):
                    nc.vector.tensor_scalar_mul(
                        qn[:, tt * 512 + h * 128: tt * 512 + (h + 1) * 128],
                        q16[:, h * 128:(h + 1) * 128],
                        ring[:, h:h + 1],
                    )
                nc.vector.tensor_scalar_mul(
                    kn[:, tt * 128:(tt + 1) * 128],
                    kv16[:, 0:128], rin[:, 4:5],
                )
                nc.scalar.copy(
                    vg[g][:, tt * 128:(tt + 1) * 128], kv16[:, 128:256],
                )

                # ---- rope on this t-tile: q [h, s, d] / k [s, d] ----
                qn4 = qn[:, tt * 512:(tt + 1) * 512].rearrange(
                    "p (h s d) -> p h s d", h=4, s=2)
                t14 = t1[:, tt * 512:(tt + 1) * 512].rearrange(
                    "p (h s d) -> p h s d", h=4, s=2)
                qf4 = qf[:, tt * 512:(tt + 1) * 512].rearrange(
                    "p (h s d) -> p h s d", h=4, s=2)
                c2t = (c2_sb[:, i * 128:(i + 1) * 128]
                       .rearrange("p (one s d) -> p one s d", one=1, s=2)
                       .broadcast_to((128, 4, 2, 64)))
                s2t = (s2_sb[:, i * 128:(i + 1) * 128]
                       .rearrange("p (one s d) -> p one s d", one=1, s=2)
                       .broadcast_to((128, 4, 2, 64)))
                nc.vector.tensor_mul(t14[:, :, 0:1, :], qn4[:, :, 1:2, :],
                                     s2t[:, :, 0:1, :])
                nc.vector.tensor_mul(t14[:, :, 1:2, :], qn4[:, :, 0:1, :],
                                     s2t[:, :, 1:2, :])
                nc.vector.tensor_mul(qf4, qn4, c2t)
                nc.vector.tensor_add(qf[:, tt * 512:(tt + 1) * 512],
                                     qf[:, tt * 512:(tt + 1) * 512],
                                     t1[:, tt * 512:(tt + 1) * 512])

                kn2 = kn[:, tt * 128:(tt + 1) * 128].rearrange(
                    "p (s d) -> p s d", s=2)
                kt2 = kt1[:, tt * 128:(tt + 1) * 128].rearrange(
                    "p (s d) -> p s d", s=2)
                kf2 = kf[:, tt * 128:(tt + 1) * 128].rearrange(
                    "p (s d) -> p s d", s=2)
                kc2 = c2_sb[:, i * 128:(i + 1) * 128].rearrange(
                    "p (s d) -> p s d", s=2)
                ks2 = s2_sb[:, i * 128:(i + 1) * 128].rearrange(
                    "p (s d) -> p s d", s=2)
                nc.vector.tensor_mul(kt2[:, 0:1, :], kn2[:, 1:2, :],
                                     ks2[:, 0:1, :])
                nc.vector.tensor_mul(kt2[:, 1:2, :], kn2[:, 0:1, :],
                                     ks2[:, 1:2, :])
                nc.vector.tensor_mul(kf2, kn2, kc2)
                nc.vector.tensor_add(kf[:, tt * 128:(tt + 1) * 128],
                                     kf[:, tt * 128:(tt + 1) * 128],
                                     kt1[:, tt * 128:(tt + 1) * 128])
                trpend.append((g, tt, i, qf[:], kf[:], qtg[:]))
        while trpend:
            emit_transposes(trpend.popleft())
        ctxB.close()

        # ================ phase C: attention (phase D interleaved) ========
        # Per-head score tiles; per j one kT/v weight load feeds both heads
        # of the pair; denominator accumulates on DVE (running sum of exp
        # tiles) with one matmul per head at the end. Phase D units for
        # query-chunk qc-1 are emitted between j iterations of chunk qc so
        # the PE stays fed while ACT runs exp (the PE queue is in-order).
        ctxC = ExitStack()
        expp = ctxC.enter_context(tc.tile_pool(name="expp", bufs=6))
        exsp = ctxC.enter_context(tc.tile_pool(name="exsp", bufs=8))
        smc = ctxC.enter_context(tc.tile_pool(name="smc", bufs=4))
        ytp = ctxC.enter_context(tc.tile_pool(name="ytp", bufs=16))
        outp = ctxC.enter_context(tc.tile_pool(name="outp", bufs=4))
        psSC = ctxC.enter_context(tc.tile_pool(name="psSC", bufs=3, space="PSUM"))
        psYT = ctxC.enter_context(tc.tile_pool(name="psYT", bufs=2, space="PSUM"))
        psFP = ctxC.enter_context(tc.tile_pool(name="psFP", bufs=2, space="PSUM"))

        def emit_d_unit(qc, tt, dc2):
            i = qc * 4 + tt
            ob = outp.tile([128, 1024], FP16, tag="ob", name=f"ob_{i}_{dc2}")
            for half in range(2):
                dc = dc2 * 2 + half
                fp = psFP.tile([128, 512], F32, tag="pFP", name=f"fp_{i}_{dc}")
                for h in range(NH_CORE):
                    nc.tensor.matmul(
                        fp[:],
                        ytile[(h, qc)][:, tt * 128:(tt + 1) * 128],
                        wp[(h, dc)][:],
                        start=(h == 0), stop=(h == 3),
                    )
                if (i + dc) % 2 == 0:
                    nc.vector.tensor_copy(ob[:, half * 512:(half + 1) * 512],
                                          fp[:])
                else:
                    nc.scalar.copy(ob[:, half * 512:(half + 1) * 512], fp[:])
            nc.sync.dma_start(
                outd[i * 128:(i + 1) * 128, dc2 * 1024:(dc2 + 1) * 1024],
                ob[:],
            )

        pending = deque()

        for qc in range(4 if "C" in phases else 0):
            jmax = 4 * qc + 3
            it = 0
            for hp in range(2):
                yt_ps = [psYT.tile([128, 512], F32, tag="pYT",
                                   name=f"yt_{qc}_{hp}_{hh}")
                         for hh in range(2)]
                exsum = exsp.tile([128, 1024], FP16, tag="exs",
                                  name=f"exs_{qc}_{hp}")
                for j in range(jmax + 1):
                    diag = j >= 4 * qc
                    dlt = 128 * j - 512 * qc
                    # in diag blocks only q-columns >= dlt survive the
                    # mask: sc/yt matmuls compute just that range (dlt is
                    # 128-aligned, so the strided qT view slices cleanly)
                    rtt = dlt // 128 if diag else 0
                    scs = [psSC.tile([128, 512], F32, tag="pSC",
                                     name=f"sc_{qc}_{hp}_{j}_{hh}")
                           for hh in range(2)]
                    for hh in range(2):
                        qv = qT[(2 * hp + hh, qc)][:].rearrange(
                            "p (tt hx d) -> p tt hx d", tt=4, hx=4)
                        nc.tensor.matmul(
                            scs[hh][:, dlt:512] if rtt else scs[hh][:],
                            kTg[j // 4][:, (j % 4) * 128:
                                        (j % 4 + 1) * 128],
                            qv[:, rtt:4, 2 * hp + hh, :],
                            start=True, stop=True,
                        )
                    # first j writes exp into the running sum tile; in diag
                    # blocks exp skips the fully-masked column prefix (the
                    # mask multiply zeroes whatever is there anyway)
                    ex = (exsum if j == 0 else
                          expp.tile([128, 1024], FP16, tag="exp",
                                    name=f"ex_{qc}_{hp}_{j}"))
                    for hh in range(2):
                        nc.scalar.activation(
                            ex[:, hh * 512 + dlt if diag else hh * 512:
                               (hh + 1) * 512],
                            scs[hh][:, dlt:512] if diag else scs[hh][:],
                            AF.Exp, bias=negb_sb[:])
                    if diag:
                        ex2 = ex[:].rearrange("p (hh q) -> p hh q", hh=2)
                        kw = (keepw_sb[:, 384 - dlt: 896 - dlt]
                              .rearrange("p (one q) -> p one q", one=1)
                              .broadcast_to((128, 2, 512)))
                        nc.vector.tensor_mul(ex2, ex2, kw)
                    for hh in range(2):
                        nc.tensor.matmul(
                            yt_ps[hh][:, dlt:512] if rtt else yt_ps[hh][:],
                            vg[j // 4][:, (j % 4) * 128:
                                      (j % 4 + 1) * 128],
                            ex[:, hh * 512 + (dlt if rtt else 0):
                               (hh + 1) * 512],
                            start=(j == 0), stop=(j == jmax))
                    if j > 0:
                        nc.vector.tensor_add(exsum[:], exsum[:], ex[:])
                    it += 1
                    if pending and it % (qc + 1) == 0:
                        emit_d_unit(*pending.popleft())
                for hh in range(2):
                    h = 2 * hp + hh
                    den_ps = psSC.tile([1, 512], F32, tag="pDB",
                                       name=f"den_{qc}_{h}", bufs=1)
                    nc.tensor.matmul(den_ps[:], onesc_sb[:],
                                     exsum[:, hh * 512:(hh + 1) * 512],
                                     start=True, stop=True)
                    rinv32 = smc.tile([1, 512], F32, tag="rinv32",
                                      name=f"rinv32_{qc}_{h}")
                    nc.vector.reciprocal_approx_fast(out=rinv32[:],
                                                     in_=den_ps[:])
                    rinv = smc.tile([1, 512], FP16, tag="rinv",
                                    name=f"rinv_{qc}_{h}")
                    nc.scalar.copy(rinv[:], rinv32[:])
                    bc_sb = smc.tile([128, 512], FP16, tag="bcs",
                                     name=f"bcs_{qc}_{h}")
                    nc.gpsimd.partition_broadcast(bc_sb[:], rinv[:])
                    yt = ytp.tile([128, 512], FP16, tag="yt",
                                  name=f"ytsb_{qc}_{h}")
                    nc.vector.tensor_mul(yt[:], yt_ps[hh][:], bc_sb[:])
                    ytile[(h, qc)] = yt
            if "D" in phases:
                for tt in range(4):
                    for dc2 in range(2):
                        pending.append((qc, tt, dc2))
        while pending:
            emit_d_unit(*pending.popleft())
        ctxC.close()

    nc.compile()
    return nc


# ---------------- host-side helpers ----------------

def rope_tables():
    inv_freq = 1.0 / (ROPE_BASE ** (np.arange(0, HD, 2, dtype=np.float32) / HD))
    t = np.arange(S, dtype=np.float32)
    fr = np.outer(t, inv_freq)
    cos = np.cos(fr).astype(np.float32)
    sin = np.sin(fr).astype(np.float32)
    c2 = np.concatenate([cos, cos], axis=1).astype(np.float16)
    s2 = np.concatenate([sin, -sin], axis=1).astype(np.float16)
    return c2, s2


def make_consts():
    c2, s2 = rope_tables()
    j = np.arange(128)[:, None]
    u = np.arange(896)[None, :]
    keepw = ((u - 384) >= j).astype(np.float16)
    ident = np.eye(128, dtype=np.float16)
    onescol = np.ones((128, 1), np.float16)
    onesrow = np.ones((1, 128), np.float16)
    return dict(c2=c2, s2=s2, keepw=keepw, ident=ident,
                onescol=onescol, onesrow=onesrow)


def make_core_inputs(x, wq, wk, wv, wproj, q_gain, core, consts=None):
    """x: [B, S, D] f32; returns in_map for `core` (0..7)."""
    if consts is None:
        consts = make_consts()
    b, g = core // 4, core % 4
    xTc = np.ascontiguousarray(x[b].T.astype(np.float16))        # [D, S]
    wqc = np.ascontiguousarray(
        wq[g * 512:(g + 1) * 512].T.astype(np.float16))          # [D, 512]
    wkc = wk[g * 128:(g + 1) * 128].T                            # [D, 128]
    wvc = wv[g * 128:(g + 1) * 128].T
    wkvc = np.ascontiguousarray(
        np.concatenate([wkc, wvc], axis=1).astype(np.float16))
    wpc = np.ascontiguousarray(
        wproj[:, g * 512:(g + 1) * 512].T.astype(np.float16))    # [512, D]
    gains = np.broadcast_to(
        (q_gain[g * 4:(g + 1) * 4] / np.sqrt(HD)).astype(np.float32)[None, :],
        (128, 4),
    ).copy()
    return dict(
        xT=xTc, wq=wqc, wkv=wkvc, wproj=wpc,
        c2=consts["c2"], s2=consts["s2"], gains=gains,
        keepw=consts["keepw"], ident=consts["ident"],
        onescol=consts["onescol"], onesrow=consts["onesrow"],
    )


# ---------------- public entry point ----------------

_PROGRAM = None


def _get_program():
    global _PROGRAM
    if _PROGRAM is None:
        _PROGRAM = build_program()
    return _PROGRAM


def kernel(x, wq, wk, wv, wproj, q_gain):
    """Causal self-attention forward. Full inputs in, full output out.

    Shards across 8 NeuronCores as 2 batches x 4 kv-head groups
    (tensor-parallel over heads); each core produces a partial output
    projection in fp16; partials are summed per batch on the host in f32
    (the unshard step for input-dim-sharded wproj).
    """
    from concourse.bass_utils import run_bass_kernel_spmd

    x = np.ascontiguousarray(np.asarray(x, dtype=np.float32))
    wq = np.ascontiguousarray(np.asarray(wq, dtype=np.float32))
    wk = np.ascontiguousarray(np.asarray(wk, dtype=np.float32))
    wv = np.ascontiguousarray(np.asarray(wv, dtype=np.float32))
    wproj = np.ascontiguousarray(np.asarray(wproj, dtype=np.float32))
    q_gain = np.asarray(q_gain, dtype=np.float32)

    nc = _get_program()
    consts = make_consts()
    in_maps = [make_core_inputs(x, wq, wk, wv, wproj, q_gain, c, consts)
               for c in range(8)]
    res = run_bass_kernel_spmd(nc, in_maps, core_ids=list(range(8)))
    parts = [r["out"].astype(np.float32) for r in res.results]
    y = np.stack([
        parts[0] + parts[1] + parts[2] + parts[3],
        parts[4] + parts[5] + parts[6] + parts[7],
    ]).astype(np.float32)
    return y
